# revision 7
# baseline (speedup 1.0000x reference)
"""Trainium2 Bass kernel for nn_CNN2_P (dense CNN + MLP head).

Pure data-parallel over 8 NeuronCores: batch 2048 -> 256 per core, all
weights replicated. Host-side prep re-tiles weights into PE-friendly
layouts and casts to bf16; the device kernel runs conv1/2/3 as
accumulating matmuls (channels on partitions) with conv3 output kept
resident in SBUF in sample-major layout (contiguous drains), then runs
fc1 "flipped": y3 sample-tiles are the stationary operand and fc1
weight rows stream from HBM as the moving operand (N=512 matmuls into
4 PSUM banks, bias folded in via a K=1 ones-matmul). fc2 is also
flipped (weights stationary, activations moving) on DMA-transposed
fc1 outputs, with a final PE transpose for the [B,16] store.
"""

import os

import numpy as np
import ml_dtypes

import concourse.mybir as mybir
import concourse.bacc as bacc
import concourse.tile as tile
from concourse.bass_utils import run_bass_kernel_spmd

# Problem constants (hardcoded per contract).
CL, IL = 128, 64          # context length, instruction length
CH = 256                  # channels in all three convs
L1, L2, L3 = 127, 125, 123
F1, OUT = 1024, 16
BATCH = 2048
NCORES = 8
SP = 128                  # y3 per-sample stride (123 data + 5 pad)

BF16 = ml_dtypes.bfloat16

_CACHE = {}


def _build_program(B_pc, G):
    """Emit the per-core Bass program. B_pc = samples per core, G = chunk."""
    bf = mybir.dt.bfloat16
    f32 = mybir.dt.float32
    nchunks = B_pc // G
    ngrp = G // 4          # 4-sample matmul groups per chunk
    NT = F1 // 128         # 8 fc1 row tiles
    NWF = 12               # wf1 stream pool depth (prefetched)
    NSB = B_pc // 128      # 128-sample blocks

    nc = bacc.Bacc("TRN2", target_bir_lowering=False, debug=False)

    xa_d = nc.dram_tensor("xa", [nchunks, 128, G * L1], bf, kind="ExternalInput")
    wa_d = nc.dram_tensor("wa", [128, CH], bf, kind="ExternalInput")
    w2_d = nc.dram_tensor("w2", [128, 12 * 128], bf, kind="ExternalInput")
    w3_d = nc.dram_tensor("w3", [128, 12 * 128], bf, kind="ExternalInput")
    wf1_d = nc.dram_tensor("wf1", [2 * L3, 128, F1], bf, kind="ExternalInput")
    wf2_d = nc.dram_tensor("wf2", [128, NT * OUT], bf, kind="ExternalInput")
    bf1_d = nc.dram_tensor("bf1", [1, F1], bf, kind="ExternalInput")
    ident_d = nc.dram_tensor("ident", [16, 16], f32, kind="ExternalInput")
    bias_d = nc.dram_tensor("bias", [128, 8], f32, kind="ExternalInput")
    out_d = nc.dram_tensor("out", [B_pc, OUT], f32, kind="ExternalOutput")

    relu = mybir.ActivationFunctionType.Relu
    ident_fn = mybir.ActivationFunctionType.Identity
    add_op = mybir.AluOpType.add
    max_op = mybir.AluOpType.max

    drain_ctr = [0]

    def drain(out_ap, in_ap, bias_ap):
        """relu(in + bias) -> out, alternating ACT / DVE."""
        if drain_ctr[0] % 2 == 0:
            nc.scalar.activation(out_ap, in_ap, relu, bias=bias_ap)
        else:
            nc.vector.tensor_scalar(out_ap, in_ap, bias_ap, 0.0, add_op, max_op)
        drain_ctr[0] += 1

    with tile.TileContext(nc) as tc:
        with tc.tile_pool(name="persist", bufs=1) as pp:
            wa_t = pp.tile([128, CH], bf, name="wa_t", tag="wa")
            nc.scalar.dma_start(out=wa_t[:], in_=wa_d.ap())
            bias_t = pp.tile([128, 8], f32, name="bias_t", tag="bias")
            nc.scalar.dma_start(out=bias_t[:], in_=bias_d.ap())
            w2_t = pp.tile([128, 12 * 128], bf, name="w2_t", tag="w2")
            nc.scalar.dma_start(out=w2_t[:], in_=w2_d.ap())
            w3_t = pp.tile([128, 12 * 128], bf, name="w3_t", tag="w3")
            nc.scalar.dma_start(out=w3_t[:], in_=w3_d.ap())
            wf2_t = pp.tile([128, NT * OUT], bf, name="wf2_t", tag="wf2")
            nc.scalar.dma_start(out=wf2_t[:], in_=wf2_d.ap())
            bf1_t = pp.tile([1, F1], bf, name="bf1_t", tag="bf1")
            nc.scalar.dma_start(out=bf1_t[:], in_=bf1_d.ap())
            ident_t = pp.tile([16, 16], f32, name="ident_t", tag="ident")
            nc.scalar.dma_start(out=ident_t[:], in_=ident_d.ap())
            ones_t = pp.tile([1, 128], bf, name="ones_t", tag="ones")
            nc.vector.memset(ones_t[:], 1.0)
            # conv3 output, resident, sample-major: y3[ct][p, s*SP + l]
            y3_t = [pp.tile([128, B_pc * SP], bf, name=f"y3_{i}", tag=f"y3_{i}")
                    for i in range(2)]
            y3v = [y3_t[i][:].rearrange("p (s l) -> p s l", l=SP) for i in range(2)]
            # fc1 output (post-relu): per 128-sample block, [s, F1]
            out1_t = [pp.tile([128, F1], bf, name=f"out1_{i}", tag=f"out1_{i}")
                      for i in range(NSB)]
            # transposed fc1 output: [f, s] per nt: cols nt*B_pc + sb*128 + s
            out1T_t = pp.tile([128, NT * B_pc], bf, name="out1T_t", tag="out1T")

            # wf1 stream pool lives across conv + fc1 phases (prefetch)
            wfp = tc.alloc_tile_pool(name="wf1", bufs=NWF)
            wf_tiles = []
            dmae = [nc.scalar, nc.gpsimd, nc.sync]
            for i in range(NWF):
                wt = wfp.tile([128, F1], bf, name="wf1_t", tag="wf1")
                dmae[i % 2].dma_start(out=wt[:], in_=wf1_d.ap()[i])
                wf_tiles.append(wt)

            # ---- conv phase ----
            with (
                tc.tile_pool(name="xa", bufs=3) as xap,
                tc.tile_pool(name="y1", bufs=2) as y1p,
                tc.tile_pool(name="y2", bufs=1) as y2p,
                tc.tile_pool(name="cpsum", bufs=8, space="PSUM") as cps,
            ):
                for c in range(nchunks):
                    xat = xap.tile([128, G * L1], bf, name="xa_t", tag="xa")
                    if c == 0:
                        q = G * L1 // 8
                        for sl in range(8):
                            nc.sync.dma_start(
                                out=xat[:, sl * q:(sl + 1) * q],
                                in_=xa_d.ap()[c][:, sl * q:(sl + 1) * q])
                    else:
                        nc.sync.dma_start(out=xat[:], in_=xa_d.ap()[c])
                    y1t = [y1p.tile([128, G * L1], bf, name=f"y1t_{i}", tag=f"y1_{i}") for i in range(2)]
                    y2t = [y2p.tile([128, G * L2], bf, name=f"y2t_{i}", tag=f"y2_{i}") for i in range(2)]
                    y1v = [y1t[i][:].rearrange("p (s l) -> p s l", l=L1)
                           for i in range(2)]
                    y2v = [y2t[i][:].rearrange("p (s l) -> p s l", l=L2)
                           for i in range(2)]

                    def emit_c1(g, ct):
                        # conv1: augmented K=128 matmul, N = 4*L1
                        ps = cps.tile([128, 4 * L1], f32, name="cps1", tag="cps")
                        nc.tensor.matmul(
                            ps[:],
                            wa_t[:, ct * 128:(ct + 1) * 128],
                            xat[:, g * 4 * L1:(g + 1) * 4 * L1],
                            start=True, stop=True,
                        )
                        drain(y1t[ct][:, g * 4 * L1:(g + 1) * 4 * L1], ps[:],
                              bias_t[:, ct:ct + 1])

                    def emit_c2(g, ct):
                        # conv2: 3x2 accumulating matmuls per (group, co_t)
                        ps = cps.tile([128, 4 * L2], f32, name="cps2", tag="cps")
                        for k in range(3):
                            for ci in range(2):
                                j = k * 4 + ci * 2 + ct
                                nc.tensor.matmul(
                                    ps[:],
                                    w2_t[:, j * 128:(j + 1) * 128],
                                    y1v[ci][:, 4 * g:4 * g + 4, k:k + L2],
                                    start=(k == 0 and ci == 0),
                                    stop=(k == 2 and ci == 1),
                                )
                        drain(y2t[ct][:, g * 4 * L2:(g + 1) * 4 * L2], ps[:],
                              bias_t[:, 2 + ct:3 + ct])

                    def emit_c3(g, ct):
                        # conv3: writes s-major into resident y3 (contiguous)
                        s0 = c * G + 4 * g
                        ps = cps.tile([128, 4 * L3], f32, name="cps3", tag="cps")
                        for k in range(3):
                            for ci in range(2):
                                j = k * 4 + ci * 2 + ct
                                nc.tensor.matmul(
                                    ps[:],
                                    w3_t[:, j * 128:(j + 1) * 128],
                                    y2v[ci][:, 4 * g:4 * g + 4, k:k + L3],
                                    start=(k == 0 and ci == 0),
                                    stop=(k == 2 and ci == 1),
                                )
                        psv = ps[:].rearrange("p (s m) -> p s m", m=L3)
                        drain(y3v[ct][:, s0:s0 + 4, 0:L3], psv[:],
                              bias_t[:, 4 + ct:5 + ct])

                    for g in range(ngrp):
                        for ct in range(2):
                            emit_c1(g, ct)
                    for g in range(ngrp):
                        for ct in range(2):
                            emit_c2(g, ct)
                    for g in range(ngrp):
                        for ct in range(2):
                            emit_c3(g, ct)

            # ---- fc1 flipped: y3 stationary, wf1 rows moving ----
            with tc.tile_pool(name="fpsum", bufs=1, space="PSUM") as fps:
                psf = [fps.tile([128, 512], f32, name=f"psf_{i}", tag=f"psf_{i}")
                       for i in range(2 * NSB)]
                # bias row via K=1 ones matmul (psf = ones.T @ bf1row)
                for sb in range(NSB):
                    for h in range(2):
                        nc.tensor.matmul(
                            psf[2 * sb + h],
                            ones_t[0:1, :],
                            bf1_t[0:1, h * 512:(h + 1) * 512],
                            start=True, stop=False,
                        )
                for ct in range(2):
                    for l in range(L3):
                        r = ct * L3 + l
                        if r < NWF:
                            wt = wf_tiles[r]
                        else:
                            wt = wfp.tile([128, F1], bf, name="wf1_t", tag="wf1")
                            dmae[r % 3].dma_start(out=wt[:], in_=wf1_d.ap()[r])
                        last = (ct == 1 and l == L3 - 1)
                        for sb in range(NSB):
                            lhsT = y3v[ct][:, sb * 128:(sb + 1) * 128, l:l + 1]
                            for h in range(2):
                                nc.tensor.matmul(
                                    psf[2 * sb + h],
                                    lhsT,
                                    wt[:, h * 512:(h + 1) * 512],
                                    start=False, stop=last,
                                )
                # relu drains: psum [s,1024] f32 -> out1 bf16
                for sb in range(NSB):
                    nc.scalar.activation(out1_t[sb][:, 0:512], psf[2 * sb], relu)
                    nc.vector.tensor_scalar(out1_t[sb][:, 512:1024],
                                            psf[2 * sb + 1], 0.0, 0.0,
                                            add_op, max_op)
            wfp.release()

            # ---- transpose out1 -> [f, s] via DMA XBAR ----
            tq = [nc.sync, nc.scalar]
            for sb in range(NSB):
                for nt in range(NT):
                    tq[(sb * NT + nt) % 2].dma_start_transpose(
                        out=out1T_t[:, nt * B_pc + sb * 128: nt * B_pc + (sb + 1) * 128],
                        in_=out1_t[sb][:, nt * 128:(nt + 1) * 128],
                    )

            # ---- fc2 flipped + bias + PE transpose + store ----
            with (
                tc.tile_pool(name="opsum", bufs=2, space="PSUM") as ops,
                tc.tile_pool(name="osb", bufs=2) as osb,
            ):
                po = ops.tile([16, B_pc], f32, name="po", tag="po")
                for nt in range(NT):
                    nc.tensor.matmul(
                        po[:],
                        wf2_t[:, nt * OUT:(nt + 1) * OUT],
                        out1T_t[:, nt * B_pc:(nt + 1) * B_pc],
                        start=(nt == 0), stop=(nt == NT - 1),
                    )
                po2 = osb.tile([16, B_pc], f32, name="po2", tag="po2")
                nc.scalar.activation(po2[:], po[:], ident_fn,
                                     bias=bias_t[0:16, 6:7])
                for sb in range(NSB):
                    pt = ops.tile([128, OUT], f32, name="pt", tag="pt")
                    nc.tensor.matmul(
                        pt[:], po2[:, sb * 128:(sb + 1) * 128], ident_t[:],
                        is_transpose=True, start=True, stop=True,
                    )
                    ot = osb.tile([128, OUT], f32, name="ot", tag="ot")
                    if sb % 2 == 0:
                        nc.scalar.copy(ot[:], pt[:])
                    else:
                        nc.vector.tensor_copy(ot[:], pt[:])
                    nc.sync.dma_start(
                        out=out_d.ap()[sb * 128:(sb + 1) * 128, :], in_=ot[:])

    nc.compile()
    return nc


def _host_prep(x, w1, b1, w2, b2, w3, b3, wfc1, bfc1, wfc2, bfc2, B_pc, G):
    """Build per-core input maps (shared weight arrays built once)."""
    NT = F1 // 128
    nchunks = B_pc // G

    # Augmented conv1 input: rows 0..63 = x0 broadcast, 64..127 = xr[:, :, 1:]
    B = x.shape[0]
    xr = np.ascontiguousarray(x.reshape(B, CL, IL).transpose(0, 2, 1))  # [B, IL, CL]
    xa = np.empty((B, 128, L1), dtype=np.float32)
    xa[:, :IL, :] = xr[:, :, 0:1]
    xa[:, IL:, :] = xr[:, :, 1:]
    xa = xa.astype(BF16)

    # conv1 weights: watilde[r, c] = w1[c, r, 0] (r<64) else w1[c, r-64, 1]
    wa = np.concatenate([w1[:, :, 0].T, w1[:, :, 1].T], axis=0).astype(BF16)
    wa = np.ascontiguousarray(wa)  # [128, 256]

    def conv_tiles(w):
        # w [co, ci, k] -> [ci(128), j*128+co], j = k*4 + ci_t*2 + co_t
        t = w.reshape(2, 128, 2, 128, 3)  # [co_t, co, ci_t, ci, k]
        t = t.transpose(4, 2, 0, 3, 1)    # [k, ci_t, co_t, ci, co]
        t = t.reshape(12, 128, 128).transpose(1, 0, 2).reshape(128, 12 * 128)
        return np.ascontiguousarray(t.astype(BF16))

    w2sb = conv_tiles(w2)
    w3sb = conv_tiles(w3)

    # fc1 weights: wf1[ct*123+l][co, n] = wfc1[n, (ct*128+co)*123+l]
    t = wfc1.reshape(F1, 2, 128, L3)      # [n, co_t, co, l]
    t = t.transpose(1, 3, 2, 0)           # [co_t, l, co, n]
    wf1 = np.ascontiguousarray(t.reshape(2 * L3, 128, F1).astype(BF16))

    # fc2: wf2[f, nt*16+o] = wfc2[o, nt*128+f]
    t = wfc2.T.reshape(NT, 128, OUT).transpose(1, 0, 2).reshape(128, NT * OUT)
    wf2 = np.ascontiguousarray(t.astype(BF16))

    bf1row = np.ascontiguousarray(bfc1.reshape(1, F1).astype(BF16))
    ident = np.ascontiguousarray(np.eye(16, dtype=np.float32))

    bias = np.zeros((128, 8), dtype=np.float32)
    bias[:, 0:2] = b1.reshape(2, 128).T
    bias[:, 2:4] = b2.reshape(2, 128).T
    bias[:, 4:6] = b3.reshape(2, 128).T
    bias[0:OUT, 6] = bfc2

    in_maps = []
    ncores = B // B_pc
    for ci in range(ncores):
        shard = xa[ci * B_pc:(ci + 1) * B_pc]            # [B_pc, 128, L1]
        shard = shard.reshape(nchunks, G, 128, L1).transpose(0, 2, 1, 3)
        shard = np.ascontiguousarray(shard).reshape(nchunks, 128, G * L1)
        in_maps.append({
            "xa": shard, "wa": wa, "w2": w2sb, "w3": w3sb,
            "wf1": wf1, "wf2": wf2, "bf1": bf1row, "ident": ident,
            "bias": bias,
        })
    return in_maps


def kernel(x, w1, b1, w2, b2, w3, b3, wfc1, bfc1, wfc2, bfc2):
    B_pc = BATCH // NCORES
    G = 16
    key = ("prog", B_pc, G)
    if key not in _CACHE:
        _CACHE[key] = _build_program(B_pc, G)
    nc = _CACHE[key]
    in_maps = _host_prep(
        np.asarray(x, dtype=np.float32), np.asarray(w1, dtype=np.float32),
        np.asarray(b1, dtype=np.float32), np.asarray(w2, dtype=np.float32),
        np.asarray(b2, dtype=np.float32), np.asarray(w3, dtype=np.float32),
        np.asarray(b3, dtype=np.float32), np.asarray(wfc1, dtype=np.float32),
        np.asarray(bfc1, dtype=np.float32), np.asarray(wfc2, dtype=np.float32),
        np.asarray(bfc2, dtype=np.float32), B_pc, G,
    )
    trace = bool(os.environ.get("KERNEL_TRACE"))
    res = run_bass_kernel_spmd(nc, in_maps, core_ids=list(range(NCORES)),
                               trace=trace)
    _CACHE["last_results"] = res
    return np.concatenate([res.results[i]["out"] for i in range(NCORES)], axis=0)
